# revision 2
# baseline (speedup 1.0000x reference)
"""Ternary-weight linear layer on 8 Trainium2 NeuronCores — two-phase.

Problem: y = x @ ternarize(W).T + b
  x [8192, 4096] fp32, W [4096, 4096] fp32, b [4096] fp32.
  ternarize(w) = round(clamp(w, -1, 1))  (round-half-even, forward value).

The dominant cost in the one-phase data-parallel kernel is reading the
replicated fp32 W (67 MB/core, 186 us at the 360 GB/s DMA roofline) for
what is 1 byte/weight of information after ternarization. Split into
two launches:

  Phase A (per core, ~34 us): DMA 1/8 of W columns ([4096, 512] fp32,
    8.4 MB), ternarize EXACTLY on the DVE in fp32 (round-half-even via
    +C/-C with C = 1.5*2^23, then clamp via min/max — round and clamp
    commute on the {-1,0,1} lattice), write fp8e4 (2.1 MB) to DRAM.
    Aggregate: W fp32 is read exactly once across the 8 cores.

  Host between phases (pure layout): concatenate the 8 fp8 slices into
    a [32 o-chunks][128 p][32 kb][128 o] blocked array so phase B's
    chunk DMAs have 2 KB contiguous runs per partition.

  Phase B (per core, ~144 us): data-parallel over tokens (1024/core).
    DMA x slice fp32 (16.8 MB) + full ternary W fp8 (16.8 MB), cast x
    to fp8e4 on the vector engine, single fp8 DoubleRow matmul pass
    (0.5 cyc/col -> 109 us PE), bias added during PSUM eviction, y out
    as bf16 (8.4 MB). x and W loads are interleaved and cells are
    emitted in data-arrival (wavefront) order so the PE ramps with the
    DMA stream instead of stalling on a strict loop nest.

  DMA busy/core: A 29 MB, B 42 MB — vs 92 MB single-phase.

Numerics: ternarize is exact, so any weight set that ternarizes to 0
yields exactly-0 output. General-case (unit-variance W) max rel err is
~2.5e-2, from the single fp8e4m3 pass over x.
"""

import numpy as np

N_CORES = 8
TOKENS = 8192
IN_F = 4096
OUT_F = 4096
T_CORE = TOKENS // N_CORES       # 1024 tokens per core
P = 128                          # partitions
KB = IN_F // P                   # 32 contraction slabs of 128
KH = KB // 2                     # 16 slabs per k-half
NQ = 8                           # x token eighths
TQ = T_CORE // NQ                # 128 tokens per eighth / per matmul
O_CORE = OUT_F // N_CORES        # 512 out columns ternarized per core (A)
OB_CORE = O_CORE // P            # 4 o-chunks produced per core (A)
N_CHUNKS = OUT_F // P            # 32 o-chunks of 128 (B)
C_ROUND = 12582912.0             # 1.5 * 2^23; (v+C)-C == round-half-even(v)

_built_w = None
_built_main = None


def _build_w():
    """Phase A: ternarize my [4096, 512] slice of W^T into fp8."""
    import concourse.bacc as bacc
    import concourse.mybir as mybir
    import concourse.tile as tile

    dt = mybir.dt

    nc = bacc.Bacc("TRN2", target_bir_lowering=False, debug=False)
    wTs_d = nc.dram_tensor("wTs", [IN_F, O_CORE], dt.float32,
                           kind="ExternalInput").ap()
    wq_d = nc.dram_tensor("wq", [P, KB, O_CORE], dt.float8e4,
                          kind="ExternalOutput").ap()

    wTs_r = wTs_d.rearrange("(kb p) o -> p kb o", p=P)   # [128, 32, 512]

    # kb-slice sizes ramp DOWN so the final in->round->clamp->out latency
    # chain operates on a small slice (the tail is latency-, not
    # throughput-bound).
    SLICES = [6, 6, 5, 5, 4, 3, 2, 1]
    assert sum(SLICES) == KB
    with tile.TileContext(nc) as tc:
        with tc.tile_pool(name="wf", bufs=3) as wfp, \
             tc.tile_pool(name="wq", bufs=4) as wqp:
            kb0 = 0
            for s, ks in enumerate(SLICES):
                wf = wfp.tile([P, ks, O_CORE], dt.float32, tag="wf",
                              name=f"wf{s}")
                nc.sync.dma_start(
                    out=wf[:], in_=wTs_r[:, kb0:kb0 + ks, :])
                # round-half-even to integer (exact in fp32 for |w|<2^22)
                nc.vector.tensor_scalar(wf[:], wf[:], C_ROUND, C_ROUND,
                                        mybir.AluOpType.add,
                                        mybir.AluOpType.subtract)
                wq = wqp.tile([P, ks, O_CORE], dt.float8e4, tag="wq",
                              name=f"wq{s}")
                # clamp to [-1, 1]; {-1, 0, 1} are exact in fp8e4
                nc.vector.tensor_scalar(wq[:], wf[:], 1.0, -1.0,
                                        mybir.AluOpType.min,
                                        mybir.AluOpType.max)
                nc.scalar.dma_start(out=wq_d[:, kb0:kb0 + ks, :], in_=wq[:])
                kb0 += ks

    nc.compile()
    return nc


# Phase-B DMA interleave: x token-eighths spread through the W-chunk
# stream (front-heavy chunk gaps, tuned against TimelineSim). The endgame
# is PE-bound, so chunks in before the last x-eighth shorten the tail;
# but chunks beyond the SBUF liveness window (WFL) are useless.
X_LAT = 2.4        # us from x DMA end to fp8 tile usable (sem + cast)
C_LAT = 2.4        # us from W DMA end to tile usable (sem prop)
CPX = 3            # W chunks between consecutive x-eighth loads
WT_BUFS = 52
YS_BUFS = 32
WFL = 22


_PLAN_GAPS = [7, 4, 3, 3, 2, 1, 1]  # chunks before each of x1..x7


def _dma_plan():
    if _PLAN_GAPS is not None:
        plan = [("c", 0), ("x", 0)]
        j = 1
        for xi, g in enumerate(_PLAN_GAPS, start=1):
            for _ in range(g):
                plan.append(("c", j))
                j += 1
            plan.append(("x", xi))
        while j < N_CHUNKS:
            plan.append(("c", j))
            j += 1
        return plan
    plan = [("x", 0)]
    xi = 1
    j = 0
    while j < N_CHUNKS:
        plan.append(("c", j))
        j += 1
        if j % CPX == 0 and xi < NQ:
            plan.append(("x", xi))
            xi += 1
    while xi < NQ:
        plan.append(("x", xi))
        xi += 1
    return plan


def _arrival_times(plan):
    """Cost-model arrival estimates (us) used to order cell emission."""
    T_XH = 2.92    # one x half-eighth DMA [128, 16, 128] f32
    T_CH = 0.73    # one W half-chunk DMA [128, 16, 128] fp8
    arr_x = {}
    arr_c = {}
    t = 0.9        # DGE lead
    for kind, i in plan:
        if kind == "x":
            t += 2 * T_XH
            arr_x[i] = t + X_LAT
        else:
            t += 2 * T_CH
            arr_c[i] = t + C_LAT
    return arr_x, arr_c


def _build_main():
    """Phase B: y^T slice = ternary-W fp8 matmul over my 1024 tokens."""
    import concourse.bacc as bacc
    import concourse.mybir as mybir
    import concourse.tile as tile

    dt = mybir.dt
    DR = mybir.MatmulPerfMode.DoubleRow

    nc = bacc.Bacc("TRN2", target_bir_lowering=False, debug=False)
    xT_d = nc.dram_tensor("xT", [IN_F, T_CORE], dt.float32,
                          kind="ExternalInput").ap()
    wq_d = nc.dram_tensor("wq", [N_CHUNKS, P, KB, P], dt.float8e4,
                          kind="ExternalInput").ap()
    biasT_d = nc.dram_tensor("biasT", [P, OUT_F // P], dt.float32,
                             kind="ExternalInput").ap()
    yT_d = nc.dram_tensor("yT", [OUT_F, T_CORE], dt.bfloat16,
                          kind="ExternalOutput").ap()

    xT_r = xT_d.rearrange("(kb p) t -> p kb t", p=P)     # [128, 32, 1024]

    plan = _dma_plan()
    arr_x, arr_c = _arrival_times(plan)

    with tile.TileContext(nc) as tc:
        with tc.tile_pool(name="xf", bufs=2) as xf, \
             tc.tile_pool(name="xq", bufs=1) as xq, \
             tc.tile_pool(name="wt", bufs=WT_BUFS) as wt, \
             tc.tile_pool(name="ys", bufs=YS_BUFS) as ys, \
             tc.tile_pool(name="cn", bufs=1) as cn, \
             tc.tile_pool(name="ps", bufs=8, space="PSUM") as ps:

            biasT = cn.tile([P, OUT_F // P], dt.float32, name="biasT_s")
            nc.sync.dma_start(out=biasT[:], in_=biasT_d[:])

            # x / W tiles per (eighth|chunk) as lists of kb-pieces:
            # [(kb0, nkb, tile), ...]. The first loads use finer pieces so
            # the PE's first matmuls start as early as possible.
            xqt = [None] * NQ
            wqt = [None] * N_CHUNKS

            def load_x(q, npieces=2):
                parts = []
                nkb = KB // npieces
                for h in range(npieces):
                    stage = xf.tile([P, nkb, TQ], dt.float32, tag="xf",
                                    name=f"xf{q}_{h}")
                    nc.sync.dma_start(
                        out=stage[:],
                        in_=xT_r[:, h * nkb:(h + 1) * nkb,
                                 q * TQ:(q + 1) * TQ])
                    t = xq.tile([P, nkb, TQ], dt.float8e4, tag=f"xq{q}{h}",
                                name=f"xq{q}_{h}")
                    # cast on the (otherwise idle) DVE so x-stage recycling
                    # never queues behind the eviction stream on ACT
                    nc.vector.tensor_scalar(t[:], stage[:], 0.0, None,
                                            mybir.AluOpType.add)
                    parts.append((h * nkb, nkb, t))
                xqt[q] = parts

            def load_c(j, npieces=2):
                parts = []
                nkb = KB // npieces
                for h in range(npieces):
                    w = wt.tile([P, nkb, P], dt.float8e4, tag="wq",
                                name=f"wq{j}_{h}")
                    nc.sync.dma_start(
                        out=w[:], in_=wq_d[j, :, h * nkb:(h + 1) * nkb, :])
                    parts.append((h * nkb, nkb, w))
                wqt[j] = parts

            def part_slice(parts, kb):
                """AP for slab-pair [kb, kb+1] within a piece list."""
                for kb0, nkb, t in parts:
                    if kb0 <= kb < kb0 + nkb:
                        o = kb - kb0
                        return t[:, o:o + 2, :]
                raise AssertionError

            fine = {}
            for kind, i in plan:
                np_ = fine.get((kind, i), 2)
                (load_x if kind == "x" else load_c)(i, npieces=np_)

            # Cells in wavefront (data-arrival) order. Each cell is 16
            # DoubleRow matmuls (full k) for (o-chunk j, token-eighth tb)
            # into its own [128, 128] PSUM tile, evicted (with bias) into
            # a half-chunk [128, 512] bf16 stage; the stage is DMA'd out
            # once its 4 cells are done (1 KB contiguous rows in yT).
            cells = sorted(
                ((max(arr_c[j], arr_x[tb]), j, tb)
                 for j in range(N_CHUNKS) for tb in range(NQ)),
                key=lambda r: (r[0], r[1], r[2]))

            # Bound chunk-tile liveness: before touching chunk j, flush any
            # remaining cells of chunks <= j - WFLUSH so the rotating wt/ys
            # pools can't form a buffer-reuse dependency cycle.
            WFLUSH = WFL
            emitted = set()
            flushed = []
            for _, j, tb in cells:
                if (j, tb) in emitted:
                    continue
                for jo in range(max(0, j - WFLUSH + 1)):
                    for tbo in range(NQ):
                        if (jo, tbo) not in emitted:
                            emitted.add((jo, tbo))
                            flushed.append((0, jo, tbo))
                emitted.add((j, tb))
                flushed.append((0, j, tb))
            cells = flushed

            ystage = {}                         # (j, tb//4) -> stage tile
            ydone = {}

            def cell(j, tb):
                g, sl = divmod(tb, NQ // 2)     # y half-chunk group, slot
                if (j, g) not in ystage:
                    ystage[(j, g)] = ys.tile([P, T_CORE // 2], dt.bfloat16,
                                             tag="y", name=f"y{j}_{g}")
                    ydone[(j, g)] = 0
                psum = ps.tile([P, TQ], dt.float32, tag="ps",
                               name=f"ps_{j}_{tb}")
                for s in range(KB // 2):
                    nc.tensor.matmul(
                        psum[:],
                        part_slice(wqt[j], 2 * s),
                        part_slice(xqt[tb], 2 * s),
                        start=(s == 0),
                        stop=(s == KB // 2 - 1),
                        perf_mode=DR)
                nc.scalar.activation(
                    ystage[(j, g)][:, sl * TQ:(sl + 1) * TQ], psum[:],
                    mybir.ActivationFunctionType.Identity,
                    bias=biasT[:, j:j + 1],
                    scale=1.0)
                ydone[(j, g)] += 1
                if ydone[(j, g)] == NQ // 2:    # half-chunk complete
                    nc.gpsimd.dma_start(
                        out=yT_d[j * P:(j + 1) * P,
                                 g * (T_CORE // 2):(g + 1) * (T_CORE // 2)],
                        in_=ystage[(j, g)][:])

            for _, j, tb in cells:
                cell(j, tb)

    nc.compile()
    return nc


def kernel(input, weight, bias):
    global _built_w, _built_main
    if _built_w is None:
        _built_w = _build_w()
    if _built_main is None:
        _built_main = _build_main()
    from concourse.bass_utils import run_bass_kernel_spmd

    input = np.ascontiguousarray(input, dtype=np.float32)
    weight = np.ascontiguousarray(weight, dtype=np.float32)
    bias = np.ascontiguousarray(bias, dtype=np.float32)

    wT = weight.T                                                # [in, out]

    # Phase A: each core ternarizes its 512 out-columns.
    in_maps_a = []
    for c in range(N_CORES):
        wTs = np.ascontiguousarray(wT[:, c * O_CORE:(c + 1) * O_CORE])
        in_maps_a.append({"wTs": wTs})
    res_a = run_bass_kernel_spmd(_built_w, in_maps_a, list(range(N_CORES)))

    # Host gather (layout only): 8 x [128 p, 32 kb, 512 o] fp8 ->
    # full blocked [32 chunks, 128 p, 32 kb, 128 o].
    wq_parts = [np.asarray(res_a.results[c]["wq"]) for c in range(N_CORES)]
    wq_full = np.ascontiguousarray(np.concatenate(
        [p.reshape(P, KB, OB_CORE, P).transpose(2, 0, 1, 3)
         for p in wq_parts], axis=0))

    biasT = np.ascontiguousarray(bias.reshape(OUT_F // P, P).T)  # [128, 32]

    in_maps_b = []
    for c in range(N_CORES):
        x_c = input[c * T_CORE:(c + 1) * T_CORE]                 # [1024, 4096]
        xT_c = np.ascontiguousarray(x_c.T)                       # [4096, 1024]
        in_maps_b.append({"xT": xT_c, "wq": wq_full, "biasT": biasT})
    res_b = run_bass_kernel_spmd(_built_main, in_maps_b,
                                 list(range(N_CORES)))

    y = np.empty((TOKENS, OUT_F), dtype=np.float32)
    for c in range(N_CORES):
        y[c * T_CORE:(c + 1) * T_CORE] = \
            np.asarray(res_b.results[c]["yT"]).astype(np.float32).T
    return y
